# revision 64
# baseline (speedup 1.0000x reference)
"""Swin-style 3D windowed attention (B=32, N=513, C=768, H=12) on 8 TRN2 cores.

Data-parallel over batch: 4 batches/core, no collectives. Fully pipelined
batch-outer structure; per batch:
  1. qkv projection with fp8e4 DoubleRow matmuls (Q/K single-term x1w1 --
     score noise tolerates it -- V 3-term residual, weights pre-scaled
     x64). Q,K land in a DoubleRow-ready fp8 layout qk8[p, cc, i] via a
     host-side column permutation of w_qkv: chunk cc holds heads
     4(cc//2)..+4 at 32-partition blocks, d-half cc%2; the psum is
     requantized to fp8 (x32) by copies alternating ACT/DVE (a heavier
     DVE share miscompiles at runtime). V in natural [token, d]
     bf16 layout with interleaved per-head ones columns.
  2. Tail pre-pass: q/k/v rows for the 513th token via free-size-1
     chains into the per-batch pq psum (u_tailchains, needs only x+w so
     it runs early); i/j/corner tail scores via fp8-DR matmuls
     (block-diagonal kt8 for j-tails) + one exp pass (u_tails2, runs
     after the qk8 copies).
  3. Attention (h-loop, software-pipelined, 2-deep): S^T scores via
     fp8-DR matmuls on 32-partition head blocks (tile_position quads)
     into [128,2,512] psum pairs; the relative-position bias is added
     IN PSUM by fp8-DR identity matmuls (stationary 64*delta(j=2p+r),
     moving fp8(16*bias) slices) so exp(x/1024) on ACT emits the final
     bf16 et directly -- no separate bias multiply on DVE/Pool. PV bf16
     with stationary [V_h | ones] giving output in [d, i] layout with
     broadcast denominators; the 513th-token PV pass runs as a
     zero-padded fp8-DR rank-1 update; recip+mul normalization on DVE.
  4. Output projection (bf16) + bias over four full 128-token tiles;
     the 513th token is projected with free-size-1 transposed matmuls
     and a scatter DMA.
Units of batch b+1 (qkv chunks ordered by head-need) and proj units of
batch b-1 are interleaved into batch b's attention emission slots; pops
are deferred to late slots in the final batch to feed the drain. A
force-drain at each batch boundary guarantees queue units are emitted
before any consumer stage.
PSUM (8 banks): "sc" attention scores (2 bufs x 2 banks), "qt" queue
units (2 x 1), "pv" main PV (1 x 1), "pq" per-batch small accums (1 x 1).
"""

import numpy as np
import ml_dtypes

import concourse.bass as bass
import concourse.mybir as mybir
import concourse.tile as tile
from concourse import bacc
from concourse.bass_utils import run_bass_kernel_spmd

B, N, C, H, Dh = 32, 513, 768, 12, 64
NCORES = 8
BC = B // NCORES
M = BC * N
KC = C // 128
BF16 = mybir.dt.bfloat16
F32 = mybir.dt.float32
FP8 = mybir.dt.float8e4
EXP = mybir.ActivationFunctionType.Exp
CPY = mybir.ActivationFunctionType.Copy
DR = mybir.MatmulPerfMode.DoubleRow
WS = 64.0   # fp8 weight pre-scale
QS = 0.5    # q/k requant: psum (x64) * 0.5 -> fp8 x32 (IEEE e4m3 max 240)
ES = 1.0 / 1024.0  # exp input scale: scores psum carries x(32*32)

_nc_cache = {}


def build_bass():
    nc = bacc.Bacc(None, target_bir_lowering=False, debug=False)

    x1d = nc.declare_dram_parameter("x1d", [C, M], FP8, isOutput=False)
    x2d = nc.declare_dram_parameter("x2d", [C, M], FP8, isOutput=False)
    w1d = nc.declare_dram_parameter("w1d", [C, 3 * C], FP8, isOutput=False)
    w2d = nc.declare_dram_parameter("w2d", [C, C], FP8, isOutput=False)
    wp = nc.declare_dram_parameter("wp", [C, C], BF16, isOutput=False)
    bp = nc.declare_dram_parameter("bp", [1, C], F32, isOutput=False)
    bpt = nc.declare_dram_parameter("bpt", [128, KC], F32, isOutput=False)
    b8d = nc.declare_dram_parameter("b8d", [128, 6, 2, 2, 1056], FP8, isOutput=False)
    i64d = nc.declare_dram_parameter("i64d", [128, 2, 128], FP8, isOutput=False)
    ebti = nc.declare_dram_parameter("ebti", [128, 48], BF16, isOutput=False)
    ebtj = nc.declare_dram_parameter("ebtj", [H, 516], BF16, isOutput=False)
    out = nc.declare_dram_parameter("out", [M, C], F32, isOutput=True)

    with tile.TileContext(nc) as tc:
        with (
            tc.tile_pool(name="persist", bufs=1) as pp,
            tc.tile_pool(name="work", bufs=2) as wk,
            tc.tile_pool(name="psum", bufs=2, space="PSUM") as ps,
        ):
            w1_sb = pp.tile([128, KC, 3 * C], FP8)
            w2v_sb = pp.tile([128, KC, C], FP8)
            wp_sb = pp.tile([128, KC, C], BF16)
            bp_sb = pp.tile([128, C], F32)
            bpt_sb = pp.tile([128, KC], F32)
            bias8_sb = pp.tile([128, 6, 2, 2, 1056], FP8)
            i64_sb = pp.tile([128, 2, 128], FP8)
            ebti_sb = pp.tile([128, 48], BF16)
            ebtj_sb = pp.tile([12, 516], BF16)

            # qk weights first: the first matmuls need only these + xtb(0)
            wr1 = w1d.rearrange("(a p) n -> p a n", p=128)
            wr2 = w2d.rearrange("(a p) n -> p a n", p=128)  # V section only
            nc.sync.dma_start(out=w1_sb[:, 0:2, 0:1536], in_=wr1[:, 0:2, 0:1536])

            def prefetch_rest():
                nc.sync.dma_start(out=i64_sb[:, :, :], in_=i64d[:, :, :])
                nc.sync.dma_start(out=bias8_sb[:, 0, :, :, :],
                                  in_=b8d[:, 0, :, :, :])
                nc.sync.dma_start(out=ebti_sb[:, :], in_=ebti[:, :])
                nc.sync.dma_start(out=ebtj_sb[:, :], in_=ebtj[:, :])

            def wp_unit():
                nc.sync.dma_start(out=wp_sb[:, :, :],
                                  in_=wp.rearrange("(a p) n -> p a n", p=128))
                nc.sync.dma_start(
                    out=bp_sb[:, :],
                    in_=bass.AP(tensor=bp, offset=0, ap=[[0, 128], [1, C]]))
                nc.sync.dma_start(out=bpt_sb[:, :], in_=bpt[:, :])

            def unit_list(b, pq):
                """Emission units for projecting batch b (qk, v, tails)."""
                col0 = b * N
                st = {}

                def u_load():
                    # 528-col rows: DoubleRow ldweights needs k-pair step %16==0
                    x1b = wk.tile([128, KC, 528], FP8, tag="x1b", bufs=2)
                    x2b = wk.tile([128, KC, 528], FP8, tag="x2b", bufs=2)
                    nc.sync.dma_start(
                        out=x1b[:, :, 0:N],
                        in_=bass.AP(tensor=x1d, offset=col0,
                                    ap=[[M, 128], [128 * M, KC], [1, N]]),
                    )
                    if b == 0:
                        for p2 in (2, 4):
                            nc.sync.dma_start(
                                out=w1_sb[:, p2:p2 + 2, 0:1536],
                                in_=wr1[:, p2:p2 + 2, 0:1536],
                            )
                    nc.sync.dma_start(
                        out=x2b[:, :, 0:N],
                        in_=bass.AP(tensor=x2d, offset=col0,
                                    ap=[[M, 128], [128 * M, KC], [1, N]]),
                    )
                    st.update(x1b=x1b, x2b=x2b)
                    et_ti = wk.tile([128, 48], BF16, tag="et_ti", bufs=2)
                    et_tails = wk.tile([12, 516], BF16, tag="et_tails", bufs=2)
                    st.update(et_ti=et_ti, et_tails=et_tails)
                    if b == 0:
                        nc.sync.dma_start(
                            out=w1_sb[:, :, 1536:2304], in_=wr1[:, :, 1536:2304]
                        )
                        nc.sync.dma_start(
                            out=w2v_sb[:, :, :], in_=wr2[:, :, :]
                        )
                    kt8 = wk.tile([128, KC, 16], FP8, tag="kt8", bufs=2)
                    nc.vector.memset(kt8[:, :, :], 0.0)
                    qk8 = wk.tile([128, 12, 528], FP8, tag="qk8", bufs=2)
                    v_sbb = wk.tile([128, 4, H, 128], BF16, tag="v_sbb", bufs=2)
                    nc.gpsimd.memset(v_sbb[:, :, :, 64:128], 1.0)
                    vt8 = wk.tile([1, 2, H, 128], FP8, tag="vt8", bufs=2)
                    nc.gpsimd.memset(vt8[0:1, 0, :, 64:128], 1.0)
                    nc.gpsimd.memset(vt8[0:1, 1, :, :], 0.0)
                    et_tails8 = wk.tile([12, 1056], FP8, tag="et_tails8",
                                        bufs=2)
                    nc.gpsimd.memset(et_tails8[:, 512:1056], 0.0)
                    st.update(kt8=kt8, qk8=qk8, v_sbb=v_sbb, vt8=vt8,
                              et_tails8=et_tails8)

                def u_qk(cc):
                    def f():
                        x1b = st["x1b"]
                        qk8 = st["qk8"]
                        # single-term Q/K: score noise tolerates it
                        terms = [(w1_sb, x1b)]
                        nt = 3 * len(terms)
                        pt = ps.tile([128, 512], F32, tag="qt", bufs=2)
                        n9 = 0
                        for p in range(KC // 2):
                            for wsb, xb in terms:
                                n9 += 1
                                nc.tensor.matmul(
                                    pt[:, :],
                                    wsb[:, 2 * p:2 * p + 2,
                                        cc * 128:(cc + 1) * 128],
                                    xb[:, 2 * p:2 * p + 2, 0:512],
                                    start=(n9 == 1), stop=(n9 == nt),
                                    perf_mode=DR,
                                )
                        if cc % 2 == 0:
                            nc.scalar.activation(out=qk8[:, cc, 0:512],
                                                 in_=pt[:, :], func=CPY,
                                                 scale=QS)
                        else:
                            nc.vector.tensor_scalar_mul(
                                qk8[:, cc, 0:512], pt[:, :], QS)
                    return f

                def u_v(jc):
                    def f():
                        x1b, x2b, v_sbb = st["x1b"], st["x2b"], st["v_sbb"]
                        terms = [(x1b, w1_sb, 1536), (x1b, w2v_sb, 0),
                                 (x2b, w1_sb, 1536)]
                        for ci, (no, nw, h0, hn) in enumerate(
                                ((0, 512, 0, 8), (512, 256, 8, 4))):
                            vv = ps.tile([128, 512], F32, tag="qt", bufs=2)
                            n9 = 0
                            for xb, wsb, o in terms:
                                for p in range(KC // 2):
                                    n9 += 1
                                    nc.tensor.matmul(
                                        vv[:, :nw],
                                        xb[:, 2 * p:2 * p + 2,
                                           jc * 128:jc * 128 + 128],
                                        wsb[:, 2 * p:2 * p + 2,
                                            o + no:o + no + nw],
                                        start=(n9 == 1), stop=(n9 == 9),
                                        perf_mode=DR,
                                    )
                            nc.vector.tensor_scalar_mul(
                                v_sbb[:, jc, h0:h0 + hn, 0:64],
                                vv[:, 0:nw], 1.0 / WS)
                    return f

                def u_tailchains():
                    # 513th-token q/k/v rows: free-size-1 chains into pq;
                    # needs only x + w, so it runs before the qk8 copies.
                    x1b, x2b = st["x1b"], st["x2b"]
                    qk8, kt8, v_sbb = st["qk8"], st["kt8"], st["v_sbb"]
                    for cc in range(12):
                        n9 = 0
                        for p in range(KC // 2):
                            n9 += 1
                            nc.tensor.matmul(
                                pq[:, 16 + cc:17 + cc],
                                w1_sb[:, 2 * p:2 * p + 2,
                                      cc * 128:(cc + 1) * 128],
                                x1b[:, 2 * p:2 * p + 2, 512:513],
                                start=(n9 == 1), stop=(n9 == 3),
                                perf_mode=DR,
                            )
                    nc.vector.tensor_scalar_mul(
                        qk8[:, 0:12, 512:513], pq[:, 16:28], QS)
                    for c in range(KC):
                        g, r = c // 2, c % 2
                        for m in range(4):
                            nc.gpsimd.tensor_copy(
                                kt8[32 * m:32 * m + 32, c,
                                    4 * g + m:4 * g + m + 1],
                                qk8[32 * m:32 * m + 32, 6 + c, 512:513],
                            )
                    # v row for the tail token (3-term)
                    terms3 = [(w1_sb, x1b, 1536), (w2v_sb, x1b, 0),
                              (w1_sb, x2b, 1536)]
                    for c6 in range(KC):
                        n9 = 0
                        for wsb, xb, o in terms3:
                            for p in range(KC // 2):
                                n9 += 1
                                nc.tensor.matmul(
                                    pq[:, 32 + c6:33 + c6],
                                    wsb[:, 2 * p:2 * p + 2,
                                        o + c6 * 128:o + (c6 + 1) * 128],
                                    xb[:, 2 * p:2 * p + 2, 512:513],
                                    start=(n9 == 1), stop=(n9 == 9),
                                    perf_mode=DR,
                                )
                    vtt = wk.tile([128, 8], FP8, tag="vtt", bufs=2)
                    nc.vector.tensor_scalar_mul(vtt[:, 0:KC],
                                                pq[:, 32:32 + KC],
                                                1.0 / WS)
                    vt8 = st["vt8"]
                    for h in range(H):
                        nc.sync.dma_start(
                            out=vt8[0:1, 0, h, 0:64],
                            in_=vtt[64 * (h % 2):64 * (h % 2) + 64,
                                    h // 2:h // 2 + 1],
                        )

                def u_tails2():
                    qk8, kt8 = st["qk8"], st["kt8"]
                    et_ti, et_tails = st["et_ti"], st["et_tails"]
                    tails = ps.tile([128, 2, 512], F32, tag="sc", bufs=2)
                    # i-tail scores: free-size-1 fp8-DR matmuls per (h, jc)
                    for h in range(H):
                        p0 = 32 * (h % 4)
                        qc2, kc2 = 2 * (h // 4), 6 + 2 * (h // 4)
                        for jc in range(4):
                            nc.tensor.matmul(
                                tails[:, 1, 4 + h * 4 + jc:5 + h * 4 + jc],
                                qk8[p0:p0 + 32, kc2:kc2 + 2,
                                    jc * 128:jc * 128 + 128],
                                qk8[p0:p0 + 32, qc2:qc2 + 2, 512:513],
                                start=True, stop=True,
                                perf_mode=DR, tile_position=(p0, 0),
                            )
                    # j-tail scores for all heads: block-diagonal kt8, fp8-DR
                    for g in range(KC // 2):
                        nc.tensor.matmul(
                            tails[0:12, 0, :], kt8[:, 2 * g:2 * g + 2, 0:12],
                            qk8[:, 2 * g:2 * g + 2, 0:512],
                            start=(g == 0), stop=(g == KC // 2 - 1),
                            perf_mode=DR,
                        )
                    for g in range(KC // 2):
                        nc.tensor.matmul(
                            tails[0:12, 1, 0:1], kt8[:, 2 * g:2 * g + 2, 0:12],
                            qk8[:, 2 * g:2 * g + 2, 512:513],
                            start=(g == 0), stop=(g == KC // 2 - 1),
                            perf_mode=DR,
                        )
                    nc.scalar.activation(
                        out=et_ti[:, :], in_=tails[:, 1, 4:52], func=EXP,
                        scale=ES,
                    )
                    nc.scalar.activation(
                        out=et_tails[:, 0:512], in_=tails[0:12, 0, :], func=EXP,
                        scale=ES,
                    )
                    nc.scalar.activation(
                        out=et_tails[:, 512:513], in_=tails[0:12, 1, 0:1],
                        func=EXP, scale=ES,
                    )
                    nc.vector.tensor_mul(et_ti[:, :], et_ti[:, :], ebti_sb[:, :])
                    nc.vector.tensor_mul(
                        st["et_tails8"][:, 0:513], et_tails[:, 0:513],
                        ebtj_sb[:, 0:513]
                    )

                units = [u_load]
                units += [u_qk(cc) for cc in (0, 1, 6, 7)]
                units += [u_v(0), u_tailchains]
                units += [u_qk(cc) for cc in (2, 3, 8, 9)]
                units += [u_v(1)]
                units += [u_qk(cc) for cc in (4, 5, 10, 11)]
                units += [u_tails2]
                units += [u_v(2), u_v(3)]
                return units, st

            def stage_a(h, st, prefetch_eb=False, pop_cb=None):
                if prefetch_eb and h % 2 == 0 and h // 2 + 1 < 6:
                    g = h // 2 + 1
                    nc.sync.dma_start(out=bias8_sb[:, g, :, :, :],
                                      in_=b8d[:, g, :, :, :])
                qk8 = st["qk8"]
                p0 = 32 * (h % 4)
                hh, hq = h % 2, h // 2
                qc2, kc2 = 2 * (h // 4), 6 + 2 * (h // 4)
                etj = wk.tile([1, 2, 528], FP8, tag="etj", bufs=4)
                nc.sync.dma_start(
                    out=etj[0:1, :, :], in_=st["et_tails8"][h:h + 1, 0:1056]
                )
                et = wk.tile([128, 4, 512], BF16, tag="et", bufs=4)
                for jcp in range(2):
                    sc = ps.tile([128, 2, 512], F32, tag="sc", bufs=2)
                    for j2 in range(2):
                        jc = 2 * jcp + j2
                        nc.tensor.matmul(
                            sc[:, j2, :],
                            qk8[p0:p0 + 32, kc2:kc2 + 2,
                                jc * 128:jc * 128 + 128],
                            qk8[p0:p0 + 32, qc2:qc2 + 2, 0:512],
                            start=True, stop=False,
                            perf_mode=DR, tile_position=(p0, 0),
                        )
                    for j2 in range(2):
                        nc.tensor.matmul(
                            sc[:, j2, :],
                            i64_sb[64 * hh:64 * hh + 64, :, :],
                            bias8_sb[64 * hh:64 * hh + 64, hq, jcp, :,
                                     j2 * 512:(j2 + 1) * 512],
                            start=False, stop=True,
                            perf_mode=DR, tile_position=(64 * hh, 0),
                        )
                    nc.scalar.activation(
                        out=et[:, 2 * jcp:2 * jcp + 2, :], in_=sc[:, :, :],
                        func=EXP, scale=ES,
                    )
                if pop_cb is not None:
                    pop_cb()
                return h, et, etj

            def stage_b(actx, st, aoTb, pq, last=False):
                h, et, etj = actx
                v_sbb, et_ti = st["v_sbb"], st["et_ti"]
                r0 = 64 * (h % 2)
                vt8 = st["vt8"]
                pv = ps.tile([128, 512], F32, tag="pv", bufs=1)
                for jc in range(4):
                    nc.tensor.matmul(
                        pv[:, :], v_sbb[:, jc, h, :], et[:, jc, :],
                        start=(jc == 0), stop=False,
                    )
                nc.tensor.matmul(
                    pv[:, :], vt8[0:1, :, h, :], etj[0:1, :, 0:512],
                    start=False, stop=True, perf_mode=DR,
                )
                for jc in range(4):
                    nc.tensor.matmul(
                        pq[:, h:h + 1], v_sbb[:, jc, h, :],
                        et_ti[:, h * 4 + jc:h * 4 + jc + 1],
                        start=(jc == 0), stop=False,
                    )
                nc.tensor.matmul(
                    pq[:, h:h + 1], vt8[0:1, 0, h, :], etj[0:1, 0, 512:513],
                    start=False, stop=True,
                )
                rc = wk.tile([64, 512], F32, tag="rc", bufs=2)
                nc.vector.reciprocal(rc[:, :], pv[64:128, :])
                nc.vector.tensor_mul(
                    aoTb[r0:r0 + 64, h // 2, 0:512], pv[0:64, :], rc[:, :]
                )
                rct = wk.tile([64, 1], F32, tag="rct", bufs=2)
                nc.vector.reciprocal(rct[:, :], pq[64:128, h:h + 1])
                nc.vector.tensor_mul(
                    aoTb[r0:r0 + 64, h // 2, 512:513], pq[0:64, h:h + 1],
                    rct[:, :]
                )

            def proj_units(b, aoTb):
                col0 = b * N
                units = []
                for mo in range(0, 512, 128):
                    mw = 128
                    shared = {}

                    def fa(mo=mo, mw=mw, shared=shared):
                        pt = ps.tile([128, 512], F32, tag="qt", bufs=2)
                        for kk in range(KC):
                            nc.tensor.matmul(
                                pt[:mw, :],
                                aoTb[:, kk, mo:mo + mw],
                                wp_sb[:, kk, 0:512],
                                start=(kk == 0), stop=(kk == KC - 1),
                            )
                        ot = wk.tile([128, 768], F32, tag="ot", bufs=4)
                        shared["ot"] = ot
                        nc.vector.tensor_add(ot[:mw, 0:512], pt[:mw, :],
                                             bp_sb[:mw, 0:512])

                    def fb(mo=mo, mw=mw, shared=shared):
                        pt = ps.tile([128, 512], F32, tag="qt", bufs=2)
                        for kk in range(KC):
                            nc.tensor.matmul(
                                pt[:mw, 0:256],
                                aoTb[:, kk, mo:mo + mw],
                                wp_sb[:, kk, 512:768],
                                start=(kk == 0), stop=(kk == KC - 1),
                            )
                        ot = shared["ot"]
                        nc.vector.tensor_add(ot[:mw, 512:768], pt[:mw, 0:256],
                                             bp_sb[:mw, 512:768])
                        nc.sync.dma_start(
                            out=out[col0 + mo:col0 + mo + mw, :], in_=ot[:mw, :]
                        )
                    units.append(fa)
                    units.append(fb)

                def ftail():
                    # 513th token: transposed free-size-1 matmuls, scatter DMA
                    pt = ps.tile([128, 512], F32, tag="qt", bufs=2)
                    for oc in range(KC):
                        for kk in range(KC):
                            nc.tensor.matmul(
                                pt[:, oc:oc + 1],
                                wp_sb[:, kk, oc * 128:oc * 128 + 128],
                                aoTb[:, kk, 512:513],
                                start=(kk == 0), stop=(kk == KC - 1),
                            )
                    ott = wk.tile([128, KC], F32, tag="ott", bufs=2)
                    nc.vector.tensor_add(ott[:, :], pt[:, 0:KC], bpt_sb[:, :])
                    nc.sync.dma_start(
                        out=bass.AP(tensor=out, offset=(col0 + 512) * C,
                                    ap=[[1, 128], [128, KC]]),
                        in_=ott[:, :],
                    )
                units.append(ftail)
                return units

            # ---- interleaved batch-pipeline driver ----
            # Queue units are emitted between attention stages; a unit's
            # products may only be consumed by stages emitted later, so at
            # each batch boundary we force-drain through the next batch's
            # projection units before its attention stages are emitted.
            pq_pre = ps.tile([128, 128], F32, tag="pq", bufs=1)
            units0, st0 = unit_list(0, pq_pre)
            for iu, u in enumerate(units0):
                u()
                if iu == 0:
                    prefetch_rest()
                    wp_unit()
            states = {0: st0}
            queue = []
            for b in range(BC):
                st = states[b]
                pq = ps.tile([128, 128], F32, tag="pq", bufs=1)
                if b + 1 < BC:
                    nunits, nst = unit_list(b + 1, pq)
                    queue.extend((b + 1, u) for u in nunits)
                    states[b + 1] = nst
                aoTb = wk.tile([128, KC, 516], BF16, tag="aoTb", bufs=2)
                pipe = []
                defer = (b == BC - 1)
                for h in range(H):
                    pipe.append(
                        stage_a(h, st, prefetch_eb=(b == 0),
                                pop_cb=(None if (defer and h < 9) else
                                        (lambda: queue.pop(0)[1]()
                                         if queue else None)))
                    )
                    if len(pipe) > 2:
                        stage_b(pipe.pop(0), st, aoTb, pq, last=defer)
                        npop = 0 if (defer and h < 9) else (1 + (h % 2 == 0))
                        for _ in range(npop):
                            if queue:
                                queue.pop(0)[1]()
                while pipe:
                    stage_b(pipe.pop(0), st, aoTb, pq, last=defer)
                    for _ in range(2):
                        if queue:
                            queue.pop(0)[1]()
                while any(tag == b + 1 for tag, _ in queue):
                    queue.pop(0)[1]()
                queue.extend((None, u) for u in proj_units(b, aoTb))
                del states[b]
            while queue:
                queue.pop(0)[1]()

    nc.compile()
    return nc


def _qk_perm():
    """Column permutation for the Q and K sections of w_qkv.

    New column cc*128 + p (cc = 2g + r) holds original channel
    h*64 + d with h = 4g + p//32, d = 32r + p%32, so that the psum of
    chunk cc lands directly in the DoubleRow layout qk8[32*(h%4)+p32,
    cc, i] with the d-dim split (p32, r)."""
    perm = np.zeros(C, np.int64)
    for cc in range(6):
        g, r = cc // 2, cc % 2
        for p in range(128):
            h = 4 * g + p // 32
            d = 32 * r + (p % 32)
            perm[cc * 128 + p] = h * 64 + d
    return perm


def _prep_inputs(x, w_qkv, w_proj, b_proj, rel_bias_table, rel_pos_index):
    bf = ml_dtypes.bfloat16
    f8 = ml_dtypes.float8_e4m3fn
    w_host = np.asarray(w_qkv, np.float32).copy()
    w_host[:, :C] *= 0.125
    w_host *= WS
    perm = _qk_perm()
    w_host[:, 0:C] = w_host[:, perm]
    w_host[:, C:2 * C] = w_host[:, C + perm]
    w1_host = w_host.astype(f8)
    w2_host = np.ascontiguousarray(
        (w_host - w1_host.astype(np.float32))[:, 2 * C:]).astype(f8)
    wp_host = np.asarray(w_proj, np.float32).astype(bf)
    bp_host = np.asarray(b_proj, np.float32).reshape(1, C)
    bpt_host = np.ascontiguousarray(
        np.asarray(b_proj, np.float32).reshape(KC, 128).T)
    g = np.asarray(rel_bias_table, np.float32)[np.asarray(rel_pos_index)]
    eb = np.exp(g).transpose(2, 0, 1)
    bb = 16.0 * g.transpose(2, 0, 1)  # [H, j, i] x16
    b8_host = np.zeros((128, 6, 2, 2, 1056), np.float32)
    blk = bb[:, :512, :512].reshape(H, 2, 2, 64, 2, 512)  # [h, jcp, j2, p, r, i]
    for h in range(H):
        hh, hq = h % 2, h // 2
        for jcp in range(2):
            for r in range(2):
                for j2 in range(2):
                    b8_host[64 * hh:64 * hh + 64, hq, jcp, r,
                            j2 * 512:(j2 + 1) * 512] = blk[h, jcp, j2, :, r, :]
    b8_host = b8_host.astype(f8)
    i64_host = np.zeros((128, 2, 128), np.float32)
    for p in range(128):
        for r in range(2):
            i64_host[p, r, 2 * (p % 64) + r] = 64.0
    i64_host = i64_host.astype(f8)
    ebti_host = np.ascontiguousarray(
        eb[:, :512, 512].reshape(H, 4, 128).transpose(2, 0, 1).reshape(128, 48)
    ).astype(bf)
    ebtj_host = np.zeros((H, 516), np.float32)
    ebtj_host[:, :513] = eb[:, 512, :]
    ebtj_host = ebtj_host.astype(bf)
    xs = np.asarray(x, np.float32).reshape(NCORES, M, C)
    in_maps = []
    for c in range(NCORES):
        xT_c = np.ascontiguousarray(xs[c].T)
        x1_c = xT_c.astype(f8)
        x2_c = (xT_c - x1_c.astype(np.float32)).astype(f8)
        in_maps.append({
            "x1d": x1_c, "x2d": x2_c, "w1d": w1_host, "w2d": w2_host,
            "wp": wp_host, "bp": bp_host, "bpt": bpt_host,
            "b8d": b8_host, "i64d": i64_host,
            "ebti": ebti_host, "ebtj": ebtj_host,
        })
    return in_maps


def run(inputs, trace=False):
    if "nc" not in _nc_cache:
        _nc_cache["nc"] = build_bass()
    nc = _nc_cache["nc"]
    in_maps = _prep_inputs(**inputs)
    res = run_bass_kernel_spmd(
        nc, in_maps, core_ids=list(range(NCORES)), trace=trace
    )
    outs = [np.asarray(r["out"], np.float32).reshape(BC, N, C)
            for r in res.results]
    return np.concatenate(outs, axis=0), res


def kernel(**inputs) -> np.ndarray:
    full, _ = run(inputs, trace=False)
    return full
